# revision 2
# baseline (speedup 1.0000x reference)
"""KMeansProbSampler Trainium2 kernel — hardware-looped, collective-free.

Key insight from profiling: on the axon/PJRT dispatch path, wall time is
dominated by per-dispatch program shipping/loading, which scales with the
static instruction count (~0.1 ms/instruction), not by HW compute (~ms) or
collectives. So v2 collapses the program with tc.For_i hardware loops
(~450 static instructions vs ~29k unrolled) and drops collectives entirely:
every core redundantly computes the FULL problem (collectives inside HW
loops do not re-execute, and unrolling 8 iterations just to host 8
collectives costs ~3k instructions => ~0.3 s of dispatch wall).

Algorithm per k-means iteration (8 total), all on one core:
  d2[p,(t,c)] for a block of 128 rows x 128 cols via one K=3 matmul per
  4-column group: lhsT = [h'^2; -2h'; 1] (rows recentered by 512), rhs row0 =
  1, row1 = a'_c, row2 = e[t,c] = a'^2_c + (w_t - b_c)^2 (+1e30 dup mask),
  with d2 = h'^2 - 2 h' a' + e = (h-a)^2 + (w-b)^2.
  m2 = min_c d2 (DVE segmented reduce); soh = (d2==m2) * (hm / max(1,sqrt(m2)))
  (value-matching argmin; first-dup-wins via the +1e30 mask on later
  duplicates, matching jnp.argmin); scatter acc[c,0:2] += soh^T @ (h_p, w_t)
  PSUM-accumulated over all 64 row/col blocks (For_i rb loop, start/stop
  bracket matmuls outside the loop).
  e, the (h,w) scatter pairs, and the per-block tables are built on device
  from tiny DRAM tables indexed by ds(rb) (no dynamic SBUF addressing).
"""

import os
import sys

import numpy as np

H = 1024
W = 1024
C = 128
N_ITER = 8
NCORES = 8
P = 128            # partitions
NBLK = 8           # column blocks (128 cols each)
NRG = 8            # row groups (128 rows each)
WG = 4             # tiles (columns) per PSUM group -> [128, 512] matmul
GPB = 4            # groups per sqrt/recip batch (16 tiles)
S_H = 512.0        # global h/a recentering
BIG = 1.0e30       # duplicate-cluster mask

_REPO_CANDIDATES = ("/opt/trn_rl_repo", "/root/.axon_site/_ro/trn_rl_repo")


def _ensure_repo():
    try:
        import concourse  # noqa: F401
        return
    except ImportError:
        pass
    for p in _REPO_CANDIDATES:
        if os.path.isdir(p):
            sys.path.insert(0, p)
            break
    import concourse  # noqa: F401


def _enable_jax_compile_cache():
    """run_bass_via_pjrt re-jits a fresh closure per dispatch; the persistent
    compilation cache turns the per-dispatch XLA recompile (~300 ms) into a
    disk hit (~10 ms)."""
    try:
        import jax
        jax.config.update("jax_compilation_cache_dir",
                          "/tmp/kmeans_jax_cache")
        jax.config.update("jax_persistent_cache_min_compile_time_secs", 0)
    except Exception:
        pass
    try:
        import jax
        jax.config.update("jax_persistent_cache_min_entry_size_bytes", -1)
    except Exception:
        pass


def build_nc(n_iter: int = N_ITER, nrg: int = NRG, nblk: int = NBLK,
             ncores: int = NCORES):
    _ensure_repo()
    import concourse.bacc as bacc
    import concourse.mybir as mybir
    import concourse.tile as tile
    from concourse.bass import ds

    f32 = mybir.dt.float32
    f32r = mybir.dt.float32r
    u8 = mybir.dt.uint8
    Alu = mybir.AluOpType
    Act = mybir.ActivationFunctionType
    X = mybir.AxisListType.X

    nrb = nrg * nblk

    nc = bacc.Bacc(
        "TRN2",
        target_bir_lowering=False,
        debug=False,
        num_devices=ncores,
    )

    # ---- I/O ----
    # heatmap: 2 pixels per byte (4-bit), unpacked on device
    hm4_d = nc.dram_tensor("hm4", [nrb, P, P // 2], u8, kind="ExternalInput")
    lhs3_d = nc.dram_tensor("lhs3", [nrb, 3, P], f32, kind="ExternalInput")
    hcol_d = nc.dram_tensor("hcol", [nrb, P], f32, kind="ExternalInput")
    wcol_d = nc.dram_tensor("wcol", [nrb, P], f32, kind="ExternalInput")
    cl_d = nc.dram_tensor("cl", [C, 2], f32, kind="ExternalInput")
    out_d = nc.dram_tensor("out", [C, 2], f32, kind="ExternalOutput")

    with tile.TileContext(nc) as tc:
        from contextlib import ExitStack

        with ExitStack() as st:
            const = st.enter_context(tc.tile_pool(name="const", bufs=1))
            work = st.enter_context(tc.tile_pool(name="work", bufs=1))
            psd = st.enter_context(tc.tile_pool(name="psd", bufs=1, space="PSUM"))
            psa = st.enter_context(tc.tile_pool(name="psa", bufs=1, space="PSUM"))
            pse = st.enter_context(tc.tile_pool(name="pse", bufs=2, space="PSUM"))
            dram = st.enter_context(tc.tile_pool(name="dram", bufs=1, space="DRAM"))

            # ---- static SBUF tiles ----
            ident = const.tile([P, P], f32)
            ltri = const.tile([P, P], f32)
            zrow = const.tile([1, P], f32)

            ncs = work.tile([C, 2], f32, name="ncs")
            apc = work.tile([C, 1], f32, name="apc")
            eqa = work.tile([C, C], f32, name="eqa")
            eqb = work.tile([C, C], f32, name="eqb")
            cfs = work.tile([C, 1], f32, name="cfs")
            a2pd = work.tile([C, 1], f32, name="a2pd")
            bbc = work.tile([C, C], f32, name="bbc")
            a2bc = work.tile([C, C], f32, name="a2bc")
            arow = work.tile([1, C], f32, name="arow")
            part = work.tile([C, 2], f32, name="part")

            rhsblk = work.tile([3, P * P], f32, name="rhsblk")
            lhs3 = work.tile([3, P], f32, name="lhs3")
            wcol = work.tile([P, 1], f32, name="wcolv")
            hcol = work.tile([P, 1], f32, name="hcolv")
            hmpk = work.tile([P, P // 2], u8, name="hmpk")
            hmblk = work.tile([P, P], u8, name="hmblk")
            u = work.tile([P, P], f32, name="u")
            ebuf = work.tile([P, P], f32, name="ebuf")
            wrow = work.tile([P, P], f32, name="wrow")
            cvhw = work.tile([P, 2 * P], f32, name="cvhw")
            sbufs = [work.tile([P, WG * P], f32, name=f"s{i}") for i in range(8)]
            m2s = [work.tile([P, GPB * WG], f32, name=f"m2_{i}") for i in range(2)]
            sqs = [work.tile([P, GPB * WG], f32, name=f"sq_{i}") for i in range(2)]
            recs = [work.tile([P, GPB * WG], f32, name=f"rec_{i}") for i in range(2)]
            sohs = [work.tile([P, P], f32, name=f"soh_{i}") for i in range(2)]

            acc = psa.tile([C, 2], f32, space="PSUM")

            cur = dram.tile([C, 2], f32)

            # ---- prologue ----
            from concourse.masks import make_identity, make_lower_triangular
            make_identity(nc, ident[:])
            make_lower_triangular(nc, ltri[:], val=1.0, diag=False)
            nc.gpsimd.dma_start(cur[:], cl_d[:])
            nc.vector.memset(zrow[:], 0.0)
            nc.vector.memset(rhsblk[:], 1.0)  # row2 stays 1; rows 0/1 rewritten

            cvhw3 = cvhw[:].rearrange("p (t two) -> p t two", two=2)

            with tc.For_i(0, n_iter) as it:
                # ---- (a) load clusters ----
                nc.gpsimd.dma_start(ncs[:], cur[:])

                # ---- (b) per-iteration tables ----
                # a' = a - 512
                nc.vector.tensor_scalar(out=apc[:], in0=ncs[:, 0:1],
                                        scalar1=S_H, scalar2=None,
                                        op0=Alu.subtract)
                # duplicate-cluster logic: cfs[i] = #earlier dups of i
                abcP = pse.tile([C, C], f32, space="PSUM", tag="bc")
                nc.tensor.transpose(out=abcP[:],
                                    in_=ncs[:, 0:1].to_broadcast([C, C]),
                                    identity=ident[:])
                nc.vector.tensor_scalar(out=eqa[:], in0=abcP[:],
                                        scalar1=ncs[:, 0:1], scalar2=None,
                                        op0=Alu.is_equal)
                bbcP = pse.tile([C, C], f32, space="PSUM", tag="bc")
                nc.tensor.transpose(out=bbcP[:],
                                    in_=ncs[:, 1:2].to_broadcast([C, C]),
                                    identity=ident[:])
                nc.vector.tensor_scalar(out=eqb[:], in0=bbcP[:],
                                        scalar1=ncs[:, 1:2], scalar2=None,
                                        op0=Alu.is_equal)
                # b values along free dim, for e-build
                nc.scalar.copy(out=bbc[:], in_=bbcP[:])
                nc.vector.tensor_tensor(out=eqa[:], in0=eqa[:], in1=eqb[:],
                                        op=Alu.mult)
                nc.vector.tensor_tensor(out=eqa[:], in0=eqa[:], in1=ltri[:],
                                        op=Alu.mult)
                nc.vector.tensor_reduce(out=cfs[:], in_=eqa[:], axis=X,
                                        op=Alu.add)
                nc.vector.tensor_scalar(out=cfs[:], in0=cfs[:], scalar1=BIG,
                                        scalar2=None, op0=Alu.mult)
                # a2pd = a'^2 + BIG*cf
                nc.vector.tensor_tensor(out=a2pd[:], in0=apc[:], in1=apc[:],
                                        op=Alu.mult)
                nc.vector.tensor_tensor(out=a2pd[:], in0=a2pd[:], in1=cfs[:],
                                        op=Alu.add)
                a2bcP = pse.tile([C, C], f32, space="PSUM", tag="bc")
                nc.tensor.transpose(out=a2bcP[:],
                                    in_=a2pd[:, 0:1].to_broadcast([C, C]),
                                    identity=ident[:])
                nc.scalar.copy(out=a2bc[:], in_=a2bcP[:])
                # rhs row1 = a'_c tiled 128x along the t axis
                apcT = pse.tile([C, C], f32, space="PSUM", tag="bc")
                nc.tensor.transpose(out=apcT[:],
                                    in_=apc[:, 0:1].to_broadcast([C, C]),
                                    identity=ident[:])
                nc.scalar.copy(out=arow[:], in_=apcT[0:1, :])
                nc.gpsimd.dma_start(rhsblk[1:2, 0:C], arow[:])
                span = C
                while span < P * P:
                    nc.gpsimd.dma_start(rhsblk[1:2, span:2 * span],
                                        rhsblk[1:2, 0:span])
                    span *= 2

                # ---- (c) PSUM zero-start for acc ----
                nc.tensor.matmul(out=acc[:], lhsT=zrow[:], rhs=zrow[0:1, 0:2],
                                 start=True, stop=False, skip_group_check=True)

                # ---- (d) block loop ----
                with tc.For_i(0, nrb) as rb:
                    nc.gpsimd.dma_start(lhs3[:], lhs3_d[ds(rb, 1), :, :])
                    nc.gpsimd.dma_start(wcol[:], wcol_d[ds(rb, 1), :])
                    nc.gpsimd.dma_start(hcol[:], hcol_d[ds(rb, 1), :])
                    nc.gpsimd.dma_start(hmpk[:], hm4_d[ds(rb, 1), :, :])
                    # unpack 4-bit pixels: t=2j low nibble, t=2j+1 high
                    hmb3 = hmblk[:].rearrange("p (j two) -> p j two", two=2)
                    nc.vector.tensor_scalar(out=hmb3[:, :, 0], in0=hmpk[:],
                                            scalar1=15, scalar2=None,
                                            op0=Alu.bitwise_and)
                    nc.vector.tensor_scalar(out=hmb3[:, :, 1], in0=hmpk[:],
                                            scalar1=4, scalar2=None,
                                            op0=Alu.logical_shift_right)

                    # e[t, c] = (b_c - w_t)^2 + a'^2_c (+ dup mask)
                    nc.vector.tensor_scalar(out=u[:], in0=bbc[:],
                                            scalar1=wcol[:], scalar2=None,
                                            op0=Alu.subtract)
                    nc.vector.tensor_tensor(out=ebuf[:], in0=u[:], in1=u[:],
                                            op=Alu.mult)
                    nc.vector.tensor_tensor(out=ebuf[:], in0=ebuf[:],
                                            in1=a2bc[:], op=Alu.add)
                    nc.gpsimd.dma_start(rhsblk[2:3, :], ebuf[:])

                    # scatter coord pairs (h_p, w_t), interleaved
                    wrowP = pse.tile([P, P], f32, space="PSUM", tag="bc")
                    nc.tensor.transpose(out=wrowP[:],
                                        in_=wcol[:, 0:1].to_broadcast([P, P]),
                                        identity=ident[:])
                    nc.scalar.copy(out=wrow[:], in_=wrowP[:])
                    # cvhw[:, 2t] = h_p: 0*ident + hcol broadcast along free
                    nc.vector.tensor_scalar(out=cvhw3[:, :, 0], in0=ident[:],
                                            scalar1=0.0, scalar2=hcol[:],
                                            op0=Alu.mult, op1=Alu.add)
                    nc.scalar.copy(out=cvhw3[:, :, 1], in_=wrow[:])

                    for g in range(P // WG):
                        q = (g // GPB) % 2
                        gg = g % GPB
                        m2 = m2s[q]
                        psum_d = psd.tile([P, WG * P], f32, space="PSUM",
                                          tag=f"psd{g % 4}")
                        nc.tensor.matmul(
                            out=psum_d[:],
                            lhsT=lhs3[:],
                            rhs=rhsblk[:, g * WG * P:(g + 1) * WG * P],
                            start=True, stop=True,
                        )
                        s = sbufs[g % 8]
                        nc.scalar.copy(out=s[:], in_=psum_d[:])
                        nc.vector.tensor_reduce(
                            out=m2[:, gg * WG:(gg + 1) * WG],
                            in_=s[:].rearrange("p (n x) -> p n x", x=P),
                            axis=X,
                            op=Alu.min,
                        )
                        if gg == GPB - 1:
                            # batched hm / max(1, sqrt(m2)) for 16 tiles
                            # hm shipped as 4-bit: fold 1/15 into the weight
                            # via sq = 15^2 * max(m2, 1)
                            sq, rec = sqs[q], recs[q]
                            nc.vector.tensor_scalar(
                                out=sq[:], in0=m2[:], scalar1=1.0,
                                scalar2=225.0, op0=Alu.max, op1=Alu.mult,
                            )
                            nc.scalar.activation(out=sq[:], in_=sq[:],
                                                 func=Act.Sqrt)
                            nc.vector.reciprocal(out=rec[:], in_=sq[:])
                            bq = g // GPB
                            nc.vector.tensor_tensor(
                                out=rec[:], in0=rec[:],
                                in1=hmblk[:, bq * 16:(bq + 1) * 16],
                                op=Alu.mult,
                            )
                            for dg in range(GPB):
                                gsrc = bq * GPB + dg
                                s_q = sbufs[gsrc % 8]
                                for tau in range(WG):
                                    t = gsrc * WG + tau
                                    col = dg * WG + tau
                                    # alternate eq between DVE and Pool to
                                    # split the per-tile [128,128] work
                                    soh = sohs[t % 2]
                                    eng = nc.vector if t % 2 == 0 else nc.gpsimd
                                    eng.tensor_scalar(
                                        out=soh[:],
                                        in0=s_q[:, tau * P:(tau + 1) * P],
                                        scalar1=m2[:, col:col + 1],
                                        scalar2=rec[:, col:col + 1],
                                        op0=Alu.is_equal,
                                        op1=Alu.mult,
                                    )
                                    nc.tensor.matmul(
                                        out=acc[:],
                                        lhsT=soh[:],
                                        rhs=cvhw3[:, t, :],
                                        start=False, stop=False,
                                        skip_group_check=True,
                                    )

                # ---- (e) close accumulation, write back ----
                nc.tensor.matmul(out=acc[:], lhsT=zrow[:], rhs=zrow[0:1, 0:2],
                                 start=False, stop=True, skip_group_check=True)
                nc.scalar.copy(out=part[:], in_=acc[:])
                nc.gpsimd.dma_start(cur[:], part[:])

            nc.gpsimd.dma_start(out_d[:], cur[:])

    nc.compile()
    return nc


def make_core_inputs(clusters: np.ndarray, heatmap: np.ndarray,
                     nrg: int = NRG, nblk: int = NBLK):
    """Host tables (identical for every core: fully redundant compute)."""
    nrb = nrg * nblk
    hmq = np.clip(np.rint(heatmap[:nrg * P, :nblk * P] * np.float32(15.0)),
                  0, 15).astype(np.uint8)
    hm4 = np.zeros((nrb, P, P // 2), np.uint8)
    lhs3 = np.zeros((nrb, 3, P), np.float32)
    hcol = np.zeros((nrb, P), np.float32)
    wcol = np.zeros((nrb, P), np.float32)
    for rg in range(nrg):
        hs = np.arange(P, dtype=np.float32) + np.float32(rg * P)
        hp = hs - np.float32(S_H)
        for b in range(nblk):
            rb = rg * nblk + b
            lhs3[rb, 0] = hp * hp
            lhs3[rb, 1] = np.float32(-2.0) * hp
            lhs3[rb, 2] = 1.0
            hcol[rb] = hs
            wcol[rb] = np.arange(P, dtype=np.float32) + np.float32(b * P)
            blkq = hmq[rg * P:(rg + 1) * P, b * P:(b + 1) * P]
            hm4[rb] = blkq[:, 0::2] | (blkq[:, 1::2] << 4)
    return {
        "hm4": hm4,
        "lhs3": lhs3,
        "hcol": hcol,
        "wcol": wcol,
        "cl": clusters.astype(np.float32),
    }


_NC_CACHE = {}


def kernel(clusters: np.ndarray, heatmap: np.ndarray) -> np.ndarray:
    _ensure_repo()
    _enable_jax_compile_cache()
    from concourse.bass_utils import run_bass_kernel_spmd

    clusters = np.asarray(clusters, np.float32)
    heatmap = np.asarray(heatmap, np.float32)

    key = (N_ITER, NRG, NBLK)
    if key not in _NC_CACHE:
        _NC_CACHE[key] = build_nc()
    nc = _NC_CACHE[key]

    im = make_core_inputs(clusters, heatmap)
    in_maps = [im for _ in range(NCORES)]
    res = run_bass_kernel_spmd(nc, in_maps, list(range(NCORES)))
    return np.asarray(res.results[0]["out"], np.float32)


if __name__ == "__main__":
    _ensure_repo()
    nc = build_nc(n_iter=int(sys.argv[1]) if len(sys.argv) > 1 else 1,
                  nrg=int(sys.argv[2]) if len(sys.argv) > 2 else 1,
                  nblk=int(sys.argv[3]) if len(sys.argv) > 3 else 1)
    print("built + compiled OK")
